# revision 1
# baseline (speedup 1.0000x reference)
"""BitfieldLinear (vq_codebook) Trainium2 kernel.

y = x @ W^T + bias with W = r[:,None]*basis[idx] + s[:,None]*(q-128)/127.

W is never materialized. Per core the two terms are computed separately
and fused in PSUM:
  - residual term: scale-folded residual weights w_res = s/127*q - 128s/127
    (|w_res| <= 0.05) decoded on ACT to bf16 [o,i], xbar-transposed, cast
    to fp8e4 [i(part), kc, o]; x is cast to fp8 in the same transposed
    layout. 16 DoubleRow (double-pumped fp8) matmuls per 512-o PSUM tile
    contract K=4096 at 2 k-chunks per pass.
  - basis term: zT[b, n] = sum_i basis[b,i]*x[n,i] via bf16 matmuls
    (K=4096 contracted once per token against only 256 basis rows), then
    Sel[b, o] = r[o]*(idx[o]==b) one-hot matmuls accumulate r*z[idx] into
    the same PSUM tile.
The fp8 quantization error of the residual path is scaled down by s
(~0.03 rms), keeping total error ~5x under the harness tolerance.

Sharding across 8 NeuronCores: 2-way over out_features (2048) x 4-way
over flattened tokens (2048). bias is added on the host (exact fp32).
"""

import numpy as np

import concourse.bass as bass
import concourse.mybir as mybir
import concourse.tile as tile
from concourse.masks import make_identity
from concourse.bass_utils import run_bass_kernel_spmd

# problem shape (hardcoded per harness contract)
B, S, D_IN, D_OUT, BASIS = 4, 2048, 4096, 4096, 256
N_CORES = 8
O_SHARDS, N_SHARDS = 2, 4           # grid: core = oc * N_SHARDS + nb
O_SH = D_OUT // O_SHARDS            # 2048 out-features per core
N_SH = (B * S) // N_SHARDS          # 2048 token rows per core

P = 128
HALF = D_IN // 2                    # 2048
KC = D_IN // P                      # 32 contraction chunks
KP = KC // 2                        # 16 DoubleRow k-pairs
NB = N_SH // P                      # 16 token blocks per core
HG = NB // 2                        # 8 half-groups (256 tokens)
NOS = O_SH // 512                   # 4 PSUM o-slices per core
OT = O_SH // P                      # 16 o-tiles per core

F32 = mybir.dt.float32
BF16 = mybir.dt.bfloat16
FP8 = mybir.dt.float8e4
I32 = mybir.dt.int32

_WAIT_LIMIT = 1


def _split_sync_waits(nc):
    """walrus in this container rejects instructions with more than one
    embedded sync-wait command; hoist the excess onto same-engine NoOps."""
    ctr = 0
    for f in nc.m.functions:
        for bb in f.blocks:
            new = []
            changed = False
            for inst in bb.instructions:
                si = inst.sync_info
                if si is not None and si.on_wait and len(si.on_wait) > _WAIT_LIMIT:
                    waits = list(si.on_wait)
                    excess, keep = waits[:-_WAIT_LIMIT], waits[-_WAIT_LIMIT:]
                    for i in range(0, len(excess), _WAIT_LIMIT):
                        ctr += 1
                        new.append(mybir.InstNoOp(
                            name=f"I-waitsplit-{ctr}",
                            engine=inst.engine,
                            ins=[], outs=[],
                            sync_info=mybir.SyncInfo(
                                on_wait=excess[i:i + _WAIT_LIMIT], on_update=[]),
                        ))
                    si.on_wait = keep
                    changed = True
                new.append(inst)
            if changed:
                bb.instructions = new


def _build_program(split_waits=True):
    nc = bass.Bass()
    Alu = mybir.AluOpType
    Act = mybir.ActivationFunctionType
    DR = mybir.MatmulPerfMode.DoubleRow

    x_in = nc.dram_tensor("x_sh", [N_SH, D_IN], F32, kind="ExternalInput")
    codes_in = nc.dram_tensor("codes_sh", [O_SH], I32, kind="ExternalInput")
    basis_in = nc.dram_tensor("basis", [BASIS, D_IN], F32, kind="ExternalInput")
    resid_in = nc.dram_tensor("resid_sh", [O_SH, D_IN], I32, kind="ExternalInput")
    scales_in = nc.dram_tensor("scales_sh", [O_SH], F32, kind="ExternalInput")
    y_out = nc.dram_tensor("y_sh", [N_SH, O_SH], F32, kind="ExternalOutput")

    with tile.TileContext(nc) as tc:
        with (
            tc.tile_pool(name="const", bufs=1) as cpool,
            tc.tile_pool(name="stg4", bufs=3) as s4pool,   # 8 KiB slots
            tc.tile_pool(name="stg2", bufs=3) as s2pool,   # 4 KiB slots
            tc.tile_pool(name="wt", bufs=2) as wtpool,
            tc.tile_pool(name="xtg", bufs=1) as xgpool,
            tc.tile_pool(name="xt8", bufs=8) as x8pool,
            tc.tile_pool(name="y", bufs=2) as ypool,
            tc.tile_pool(name="psmm", bufs=4, space="PSUM") as mmpool,
            tc.tile_pool(name="pszt", bufs=2, space="PSUM") as ztpool,
            tc.tile_pool(name="psrt", bufs=2, space="PSUM") as rtpool,
        ):
            # ---- persistent tensors --------------------------------
            resid8T = cpool.tile([P, KC, O_SH], FP8, name="resid8T")
            basisT = cpool.tile([P, KC, BASIS], BF16, name="basisT")
            sel_sb = cpool.tile([P, 2, O_SH], BF16, name="sel_sb")
            zT_all = cpool.tile([P, 2, N_SH], BF16, name="zT_all")

            # ---- decode code rows ----------------------------------
            codes_row = s4pool.tile([1, O_SH], I32, tag="s4", name="codes_row")
            nc.sync.dma_start(codes_row[:], codes_in[None, :])
            idx_tmp = s4pool.tile([1, O_SH], I32, tag="s4", name="idx_tmp")
            nc.vector.tensor_scalar(idx_tmp[:], codes_row[:], 0xFF, None,
                                    Alu.bitwise_and)
            idx_row_f = s2pool.tile([1, O_SH], BF16, tag="s2", name="idx_row")
            nc.scalar.activation(idx_row_f[:], idx_tmp[:], Act.Copy)
            rq_row = s4pool.tile([1, O_SH], I32, tag="s4", name="rq_row")
            nc.vector.tensor_scalar(rq_row[:], codes_row[:], 8, 0xFFFF,
                                    Alu.logical_shift_right, Alu.bitwise_and)
            r_row_f = s2pool.tile([1, O_SH], BF16, tag="s2", name="r_row")
            nc.scalar.activation(r_row_f[:], rq_row[:], Act.Copy,
                                 scale=1.0 / 65535.0)

            # per-o-tile decode scalars: s/127 (scale), -128*s/127 (bias)
            s_pp = cpool.tile([P, OT], F32)
            nc.gpsimd.dma_start(s_pp[:], scales_in.rearrange("(t p) -> p t", p=P))
            sv_pp = cpool.tile([P, OT], F32)
            nc.vector.tensor_scalar_mul(sv_pp[:], s_pp[:], 1.0 / 127.0)
            bv_pp = cpool.tile([P, OT], F32)
            nc.vector.tensor_scalar_mul(bv_pp[:], s_pp[:], -128.0 / 127.0)

            ones_row = cpool.tile([1, P], BF16)
            nc.vector.memset(ones_row[:], 1.0)
            identity = cpool.tile([P, P], BF16)
            make_identity(nc, identity[:])

            # ---- one-hot Sel [128 b_lo, 2 b_hi, o]: r[o]@row idx[o] --
            iota_i = cpool.tile([P, 1], I32)
            nc.gpsimd.iota(iota_i[:], pattern=[[0, 1]], base=0,
                           channel_multiplier=1)
            iota_f = [cpool.tile([P, 1], F32, name=f"iota_f{bh}")
                      for bh in range(2)]
            nc.scalar.activation(iota_f[0][:], iota_i[:], Act.Copy)
            nc.scalar.activation(iota_f[1][:], iota_i[:], Act.Copy, bias=128.0,
                                 scale=1.0)
            for q in range(NOS):
                qs = slice(q * 512, (q + 1) * 512)
                pr = rtpool.tile([P, 512], F32, tag="rt", name=f"pr{q}")
                nc.tensor.matmul(pr[:], lhsT=ones_row[:], rhs=r_row_f[:, qs],
                                 start=True, stop=True)
                r_bc = ypool.tile([P, 512], BF16, tag="y", name=f"rbc{q}")
                nc.scalar.copy(r_bc[:], pr[:])
                pi = rtpool.tile([P, 512], F32, tag="rt", name=f"pi{q}")
                nc.tensor.matmul(pi[:], lhsT=ones_row[:], rhs=idx_row_f[:, qs],
                                 start=True, stop=True)
                for bh in range(2):
                    # (idx - 128*bh == iota) * r
                    nc.vector.scalar_tensor_tensor(
                        sel_sb[:, bh, qs], pi[:], iota_f[bh][:, :1], r_bc[:],
                        op0=Alu.is_equal, op1=Alu.mult)

            # ---- basisT [128 i_lo, kc, b] --------------------------
            for bh in range(2):
                for hf in range(2):
                    hs = slice(hf * HALF, (hf + 1) * HALF)
                    b_nat = s4pool.tile([P, HALF], F32, tag="s4",
                                        name=f"bnat{bh}_{hf}")
                    nc.gpsimd.dma_start(b_nat[:], basis_in[bh * P:(bh + 1) * P, hs])
                    b_bf = s2pool.tile([P, HALF], BF16, tag="s2",
                                       name=f"bbf{bh}_{hf}")
                    nc.vector.tensor_copy(b_bf[:], b_nat[:])
                    nc.scalar.dma_start_transpose(
                        basisT[:, hf * KP:(hf + 1) * KP, bh * P:(bh + 1) * P],
                        b_bf[:])

            # ---- resid build: o-tile t -> resid8T[:, :, t*128...] ---
            def resid_tiles(ts_range):
                for t in ts_range:
                    for hf in range(2):
                        hs = slice(hf * HALF, (hf + 1) * HALF)
                        r_nat = s4pool.tile([P, HALF], I32, tag="s4",
                                            name=f"rnat{t}_{hf}")
                        reng = nc.gpsimd if (t + hf) % 2 == 0 else nc.sync
                        reng.dma_start(r_nat[:], resid_in[t * P:(t + 1) * P, hs])
                        # w_res = s/127 * q - 128*s/127  (per-partition o)
                        w_bf = s2pool.tile([P, HALF], BF16, tag="s2",
                                           name=f"wbf{t}_{hf}")
                        nc.scalar.activation(w_bf[:], r_nat[:], Act.Identity,
                                             bias=bv_pp[:, t:t + 1],
                                             scale=sv_pp[:, t:t + 1])
                        wT = wtpool.tile([P, KP, P], BF16, tag="wt",
                                         name=f"wT{t}_{hf}")
                        nc.sync.dma_start_transpose(wT[:], w_bf[:])
                        nc.vector.tensor_copy(
                            resid8T[:, hf * KP:(hf + 1) * KP,
                                    t * P:(t + 1) * P], wT[:])

            # ---- resid build via PE transposes (ramp tiles) ---------
            # during the startup window the tensor engine is idle while
            # resid/x stream in; transposing the first o-tiles there both
            # fills the bubble and unloads the xbar transpose ring
            def resid_tiles_pe(ts_range):
                for t in ts_range:
                    w_hf = []
                    for hf in range(2):
                        hs = slice(hf * HALF, (hf + 1) * HALF)
                        r_nat = s4pool.tile([P, HALF], I32, tag="s4",
                                            name=f"rnat{t}_{hf}")
                        reng = nc.gpsimd if (t + hf) % 2 == 0 else nc.sync
                        reng.dma_start(r_nat[:], resid_in[t * P:(t + 1) * P, hs])
                        w_bf = s2pool.tile([P, HALF], BF16, tag="s2",
                                           name=f"wbf{t}_{hf}")
                        nc.scalar.activation(w_bf[:], r_nat[:], Act.Identity,
                                             bias=bv_pp[:, t:t + 1],
                                             scale=sv_pp[:, t:t + 1])
                        w_hf.append(w_bf)
                    for m in range(KC // 4):
                        prt = rtpool.tile([P, 512], BF16, tag="rt",
                                          name=f"prt{t}_{m}")
                        for j in range(4):
                            kc = 4 * m + j
                            src = w_hf[kc // KP]
                            ks = slice((kc % KP) * P, (kc % KP + 1) * P)
                            nc.tensor.transpose(prt[:, j * P:(j + 1) * P],
                                                src[:, ks], identity[:])
                        nc.vector.tensor_copy(
                            resid8T[:, 4 * m:4 * m + 4, t * P:(t + 1) * P],
                            prt[:])

            # ---- x pipeline: half-group hg = blocks 2hg, 2hg+1 ------
            xt8 = {}

            def x_halfgroup(hg):
                xtg = xgpool.tile([P, KC, 2 * P], BF16, tag="xtg",
                                  name=f"xtg{hg}")
                for j in range(2):
                    nb = hg * 2 + j
                    for hf in range(2):
                        hs = slice(hf * HALF, (hf + 1) * HALF)
                        x_nat = s4pool.tile([P, HALF], F32, tag="s4",
                                            name=f"xnat{nb}_{hf}")
                        xeng = nc.sync if (nb + hf) % 2 == 0 else nc.gpsimd
                        xeng.dma_start(x_nat[:], x_in[nb * P:(nb + 1) * P, hs])
                        x_bf = s2pool.tile([P, HALF], BF16, tag="s2",
                                           name=f"xbf{nb}_{hf}")
                        nc.vector.tensor_copy(x_bf[:], x_nat[:])
                        nc.scalar.dma_start_transpose(
                            xtg[:, hf * KP:(hf + 1) * KP, j * P:(j + 1) * P],
                            x_bf[:])
                    x8 = x8pool.tile([P, KC, P], FP8, tag="x8", name=f"x8_{nb}")
                    nc.scalar.copy(x8[:], xtg[:, :, j * P:(j + 1) * P])
                    xt8[nb] = x8
                # zT[b, n] for this half-group's 256 tokens
                for bt in range(2):
                    psz = ztpool.tile([P, 2 * P], F32, tag="zt",
                                      name=f"psz{hg}_{bt}")
                    for kc in range(KC):
                        nc.tensor.matmul(psz[:],
                                         lhsT=basisT[:, kc, bt * P:(bt + 1) * P],
                                         rhs=xtg[:, kc, :],
                                         start=(kc == 0), stop=(kc == KC - 1))
                    nc.scalar.copy(zT_all[:, bt, hg * 2 * P:(hg + 1) * 2 * P],
                                   psz[:])

            # ---- main unit: y[nb-block, os-slice] -------------------
            def unit(nb, os):
                osl = slice(os * 512, (os + 1) * 512)
                ps = mmpool.tile([P, 512], F32, tag="mm", name=f"ps{nb}_{os}")
                for kp in range(KP):
                    nc.tensor.matmul(ps[:],
                                     lhsT=xt8[nb][:, 2 * kp:2 * kp + 2, :],
                                     rhs=resid8T[:, 2 * kp:2 * kp + 2, osl],
                                     start=(kp == 0), stop=False,
                                     perf_mode=DR)
                for bt in range(2):
                    nc.tensor.matmul(ps[:],
                                     lhsT=zT_all[:, bt, nb * P:(nb + 1) * P],
                                     rhs=sel_sb[:, bt, osl],
                                     start=False, stop=(bt == 1))
                y_t = ypool.tile([P, 512], F32, tag="y", name=f"y{nb}_{os}")
                nc.vector.tensor_copy(y_t[:], ps[:])
                nc.scalar.dma_start(y_out[nb * P:(nb + 1) * P, osl], y_t[:])

            # ---- emission: interleaved to match data-arrival order --
            resid_tiles_pe(range(0, 4))
            x_halfgroup(0)
            x_halfgroup(1)
            resid_tiles_pe(range(4, 8))
            x_halfgroup(2)
            x_halfgroup(3)
            for nb in range(0, 8):
                unit(nb, 0)
            resid_tiles(range(8, 12))
            for nb in range(0, 8):
                unit(nb, 1)
            resid_tiles(range(12, 16))
            for nb in range(0, 8):
                unit(nb, 2)
            for nb in range(0, 8):
                unit(nb, 3)
            # pass B: second half of the token blocks
            x_halfgroup(4)
            x_halfgroup(5)
            x_halfgroup(6)
            x_halfgroup(7)
            for os in range(NOS):
                for nb in range(8, 16):
                    unit(nb, os)

    if split_waits:
        _split_sync_waits(nc)
    return nc


_program_cache = {}


def _get_program():
    if "nc" not in _program_cache:
        _program_cache["nc"] = _build_program()
    return _program_cache["nc"]


def kernel(x, codes, basis_table, residual_q, residual_scales, bias):
    x = np.ascontiguousarray(np.asarray(x, dtype=np.float32))
    codes = np.ascontiguousarray(np.asarray(codes, dtype=np.int32))
    basis_table = np.ascontiguousarray(np.asarray(basis_table, dtype=np.float32))
    residual_q = np.ascontiguousarray(np.asarray(residual_q, dtype=np.int32))
    residual_scales = np.ascontiguousarray(
        np.asarray(residual_scales, dtype=np.float32))
    bias = np.ascontiguousarray(np.asarray(bias, dtype=np.float32))

    x2 = x.reshape(B * S, D_IN)
    in_maps = []
    for core in range(N_CORES):
        oc, nb = divmod(core, N_SHARDS)
        osl = slice(oc * O_SH, (oc + 1) * O_SH)
        nsl = slice(nb * N_SH, (nb + 1) * N_SH)
        in_maps.append({
            "x_sh": np.ascontiguousarray(x2[nsl]),
            "codes_sh": np.ascontiguousarray(codes[osl]),
            "basis": basis_table,
            "resid_sh": np.ascontiguousarray(residual_q[osl]),
            "scales_sh": np.ascontiguousarray(residual_scales[osl]),
        })

    nc = _get_program()
    res = run_bass_kernel_spmd(nc, in_maps, core_ids=list(range(N_CORES)))

    y = np.empty((B * S, D_OUT), dtype=np.float32)
    for core in range(N_CORES):
        oc, nb = divmod(core, N_SHARDS)
        y[nb * N_SH:(nb + 1) * N_SH, oc * O_SH:(oc + 1) * O_SH] = \
            res.results[core]["y_sh"]
    y += bias[None, :]
    return y.reshape(B, S, D_OUT)



# revision 2
# speedup vs baseline: 1.4579x; 1.4579x over previous
"""BitfieldLinear (vq_codebook) Trainium2 kernel — yT formulation.

y = x @ W^T + bias with W = r[:,None]*basis[idx] + s[:,None]*(q-128)/127.

Each core computes yT[o, n] (out-features on PSUM partitions) so the
per-o decode scale applies per-partition AFTER the matmul:
  yT = (s/127) * [x @ (q-128)^T]^T  +  Sel @ z            (+ bias on host)
       with z[b, n] = sum_i basis[b, i] x[n, i],
       Sel[b, o] = r[o] * (idx[o] == b).
The host ships transposed, pre-packed operands (xT bf16, (q-128)T int8 —
a lossless bit repack of the int32 input — basisT bf16), so the device
does no transposes and no weight decode pass: the int8 residual is cast
to fp8e4 (ACT), x to fp8, and 16 DoubleRow fp8 matmuls per [128o x 512n]
PSUM tile contract K=4096 at the fp8 peak. The basis term accumulates in
a second PSUM bank via bf16 one-hot matmuls; a per-partition-scaled ACT
evacuation + DVE add fuse the two terms.

Sharding across 8 NeuronCores: 2-way over out_features (2048) x 4-way
over flattened tokens (2048). bias is added on the host (exact fp32).
"""

import numpy as np
from ml_dtypes import bfloat16

import concourse.bass as bass
import concourse.mybir as mybir
import concourse.tile as tile
from concourse.bass_utils import run_bass_kernel_spmd

# problem shape (hardcoded per harness contract)
B, S, D_IN, D_OUT, BASIS = 4, 2048, 4096, 4096, 256
N_CORES = 8
O_SHARDS, N_SHARDS = 2, 4           # grid: core = oc * N_SHARDS + nb
O_SH = D_OUT // O_SHARDS            # 2048 out-features per core
N_SH = (B * S) // N_SHARDS          # 2048 token rows per core

P = 128
KC = D_IN // P                      # 32 contraction chunks
KP = KC // 2                        # 16 DoubleRow k-pairs
OT = O_SH // P                      # 16 o-blocks per core
NSL = 4                             # token slices per core
NW = N_SH // NSL                    # 512 tokens per slice
HN = NW // 2                        # 256-token half-slices for x loads

F32 = mybir.dt.float32
BF16 = mybir.dt.bfloat16
FP8 = mybir.dt.float8e4
I32 = mybir.dt.int32
I8 = mybir.dt.int8

_WAIT_LIMIT = 1


def _split_sync_waits(nc):
    """walrus in this container rejects instructions with more than one
    embedded sync-wait command; hoist the excess onto same-engine NoOps."""
    ctr = 0
    for f in nc.m.functions:
        for bb in f.blocks:
            new = []
            changed = False
            for inst in bb.instructions:
                si = inst.sync_info
                if si is not None and si.on_wait and len(si.on_wait) > _WAIT_LIMIT:
                    waits = list(si.on_wait)
                    excess, keep = waits[:-_WAIT_LIMIT], waits[-_WAIT_LIMIT:]
                    for i in range(0, len(excess), _WAIT_LIMIT):
                        ctr += 1
                        new.append(mybir.InstNoOp(
                            name=f"I-waitsplit-{ctr}",
                            engine=inst.engine,
                            ins=[], outs=[],
                            sync_info=mybir.SyncInfo(
                                on_wait=excess[i:i + _WAIT_LIMIT], on_update=[]),
                        ))
                    si.on_wait = keep
                    changed = True
                new.append(inst)
            if changed:
                bb.instructions = new


def _build_program(split_waits=True):
    nc = bass.Bass()
    Alu = mybir.AluOpType
    Act = mybir.ActivationFunctionType
    DR = mybir.MatmulPerfMode.DoubleRow

    # packed layouts (host-side):
    #   xp[p, ns, h, kc*HN + n] = x[ns*NW + h*HN + n, kc*P + p]   (bf16)
    #   qp[p, g, kc*P + o]      = q[g*P + o, kc*P + p] - 128      (int8)
    #   bp[p, kc, b]            = basis[b, kc*P + p]              (bf16)
    x_in = nc.dram_tensor("xp", [P, NSL, 2, KC * HN], BF16, kind="ExternalInput")
    q_in = nc.dram_tensor("qp", [P, OT, KC * P], I8, kind="ExternalInput")
    b_in = nc.dram_tensor("bp", [P, KC, BASIS], BF16, kind="ExternalInput")
    codes_in = nc.dram_tensor("codes_sh", [O_SH], I32, kind="ExternalInput")
    scales_in = nc.dram_tensor("scales_sh", [O_SH], F32, kind="ExternalInput")
    y_out = nc.dram_tensor("y_sh", [O_SH, N_SH], BF16, kind="ExternalOutput")

    with tile.TileContext(nc) as tc:
        with (
            tc.tile_pool(name="const", bufs=1) as cpool,
            tc.tile_pool(name="rows4", bufs=2) as r4pool,   # [1, O_SH] i32
            tc.tile_pool(name="rows2", bufs=2) as r2pool,   # [1, O_SH] bf16
            tc.tile_pool(name="xbf", bufs=2) as xbfpool,
            tc.tile_pool(name="x8", bufs=2) as x8pool,
            tc.tile_pool(name="qst", bufs=3) as qstpool,
            tc.tile_pool(name="zsb", bufs=2) as zsbpool,
            tc.tile_pool(name="rsb", bufs=3) as rsbpool,
            tc.tile_pool(name="y", bufs=4) as ypool,
            tc.tile_pool(name="psres", bufs=3, space="PSUM") as respool,
            tc.tile_pool(name="psbas", bufs=3, space="PSUM") as baspool,
            tc.tile_pool(name="psz", bufs=2, space="PSUM") as zpool,
        ):
            # ---- persistent tensors --------------------------------
            qT8 = cpool.tile([P, OT, KC, P], FP8, name="qT8")
            basisT = cpool.tile([P, KC, BASIS], BF16, name="basisT")
            sel_sb = cpool.tile([P, 2, O_SH], BF16, name="sel_sb")

            # ---- decode code rows ----------------------------------
            codes_row = r4pool.tile([1, O_SH], I32, tag="r4", name="codes_row")
            nc.scalar.dma_start(codes_row[:], codes_in[None, :])
            idx_tmp = r4pool.tile([1, O_SH], I32, tag="r4", name="idx_tmp")
            nc.vector.tensor_scalar(idx_tmp[:], codes_row[:], 0xFF, None,
                                    Alu.bitwise_and)
            idx_row_f = r2pool.tile([1, O_SH], BF16, tag="r2", name="idx_row")
            nc.scalar.activation(idx_row_f[:], idx_tmp[:], Act.Copy)
            rq_row = r4pool.tile([1, O_SH], I32, tag="r4", name="rq_row")
            nc.vector.tensor_scalar(rq_row[:], codes_row[:], 8, 0xFFFF,
                                    Alu.logical_shift_right, Alu.bitwise_and)
            r_row_f = r2pool.tile([1, O_SH], BF16, tag="r2", name="r_row")
            nc.scalar.activation(r_row_f[:], rq_row[:], Act.Copy,
                                 scale=1.0 / 65535.0)

            # per-o-block decode scale s/127 (per-partition in yT layout)
            s_pp = cpool.tile([P, OT], F32)
            nc.gpsimd.dma_start(s_pp[:], scales_in.rearrange("(g p) -> p g", p=P))
            sv_pp = cpool.tile([P, OT], F32)
            nc.vector.tensor_scalar_mul(sv_pp[:], s_pp[:], 1.0 / 127.0)

            ones_row = cpool.tile([1, P], BF16)
            nc.vector.memset(ones_row[:], 1.0)

            # ---- one-hot Sel [128 b_lo, 2 b_hi, o]: r[o]@row idx[o] --
            iota_i = cpool.tile([P, 1], I32)
            nc.gpsimd.iota(iota_i[:], pattern=[[0, 1]], base=0,
                           channel_multiplier=1)
            iota_f = [cpool.tile([P, 1], F32, name=f"iota_f{bh}")
                      for bh in range(2)]
            nc.scalar.activation(iota_f[0][:], iota_i[:], Act.Copy)
            nc.scalar.activation(iota_f[1][:], iota_i[:], Act.Copy, bias=128.0,
                                 scale=1.0)
            for q in range(O_SH // 512):
                qs = slice(q * 512, (q + 1) * 512)
                pr = zpool.tile([P, 512], F32, tag="zt", name=f"pr{q}")
                nc.tensor.matmul(pr[:], lhsT=ones_row[:], rhs=r_row_f[:, qs],
                                 start=True, stop=True)
                r_bc = ypool.tile([P, 512], BF16, tag="y", name=f"rbc{q}")
                nc.scalar.copy(r_bc[:], pr[:])
                pi = zpool.tile([P, 512], F32, tag="zt", name=f"pi{q}")
                nc.tensor.matmul(pi[:], lhsT=ones_row[:], rhs=idx_row_f[:, qs],
                                 start=True, stop=True)
                for bh in range(2):
                    # (idx - 128*bh == iota) * r
                    nc.vector.scalar_tensor_tensor(
                        sel_sb[:, bh, qs], pi[:], iota_f[bh][:, :1], r_bc[:],
                        op0=Alu.is_equal, op1=Alu.mult)

            # ---- basisT load (pre-packed) --------------------------
            nc.scalar.dma_start(basisT[:], b_in[:])

            # ---- q chunk pipeline: DMA (gpsimd) + fp8 cast (ACT) ---
            def q_chunk(g):
                qst = qstpool.tile([P, 1, KC, P], I8, tag="qst", name=f"qst{g}")
                nc.gpsimd.dma_start(qst[:], q_in[:, g:g + 1, :])
                nc.scalar.copy(qT8[:, g:g + 1], qst[:])

            # ---- x loads + casts -----------------------------------
            xbf = {}     # (ns, h) -> bf16 half-slice tile
            x8 = {}      # ns -> fp8 slice tile

            def x_load(ns, h):
                t = xbfpool.tile([P, KC, HN], BF16, tag="xbf",
                                 name=f"xbf{ns}_{h}")
                nc.sync.dma_start(t[:], x_in[:, ns:ns + 1, h:h + 1, :])
                xbf[(ns, h)] = t

            def x_cast(ns):
                t = x8pool.tile([P, KC, NW], FP8, tag="x8", name=f"x8_{ns}")
                for h in range(2):
                    nc.gpsimd.tensor_copy(t[:, :, h * HN:(h + 1) * HN],
                                          xbf[(ns, h)][:])
                x8[ns] = t

            # ---- z matmuls: z[b, n-slice] --------------------------
            def z_slice(ns):
                psz = [zpool.tile([P, NW], F32, tag="zt", name=f"psz{ns}_{bt}")
                       for bt in range(2)]
                for h in range(2):
                    hs = slice(h * HN, (h + 1) * HN)
                    for bt in range(2):
                        for kc in range(KC):
                            nc.tensor.matmul(
                                psz[bt][:, hs],
                                lhsT=basisT[:, kc, bt * P:(bt + 1) * P],
                                rhs=xbf[(ns, h)][:, kc, :],
                                start=(kc == 0), stop=(kc == KC - 1))
                z_sb = zsbpool.tile([P, 2, NW], BF16, tag="zsb",
                                    name=f"zsb{ns}")
                for bt in range(2):
                    nc.vector.tensor_copy(z_sb[:, bt, :], psz[bt][:])
                return z_sb

            # ---- main unit: yT[o-block g, n-slice ns] --------------
            def unit(ns, g, z_sb):
                ps = respool.tile([P, NW], F32, tag="res", name=f"ps{ns}_{g}")
                for kp in range(KP):
                    nc.tensor.matmul(ps[:],
                                     lhsT=qT8[:, g, 2 * kp:2 * kp + 2, :],
                                     rhs=x8[ns][:, 2 * kp:2 * kp + 2, :],
                                     start=(kp == 0), stop=(kp == KP - 1),
                                     perf_mode=DR)
                pb = baspool.tile([P, NW], F32, tag="bas", name=f"pb{ns}_{g}")
                for bt in range(2):
                    nc.tensor.matmul(pb[:],
                                     lhsT=sel_sb[:, bt, g * P:(g + 1) * P],
                                     rhs=z_sb[:, bt, :],
                                     start=(bt == 0), stop=(bt == 1))
                # yT = (s/127) * ps + pb
                r_sb = rsbpool.tile([P, NW], BF16, tag="rsb",
                                    name=f"rs{ns}_{g}")
                nc.scalar.activation(r_sb[:], ps[:], Act.Copy,
                                     scale=sv_pp[:, g:g + 1])
                y_t = ypool.tile([P, NW], BF16, tag="y", name=f"y{ns}_{g}")
                nc.vector.tensor_tensor(y_t[:], r_sb[:], pb[:], Alu.add)
                yeng = nc.sync if g % 2 == 0 else nc.gpsimd
                yeng.dma_start(
                    y_out[g * P:(g + 1) * P, ns * NW:(ns + 1) * NW], y_t[:])

            # ---- emission -----------------------------------------
            for g in range(4):
                q_chunk(g)
            x_load(0, 0)
            x_load(0, 1)
            qg = 4
            for ns in range(NSL):
                if ns + 1 < NSL:
                    x_load(ns + 1, 0)
                    x_load(ns + 1, 1)
                x_cast(ns)
                z_sb = z_slice(ns)
                for g in range(OT):
                    if qg < OT:
                        q_chunk(qg)
                        qg += 1
                    unit(ns, g, z_sb)

    if split_waits:
        _split_sync_waits(nc)
    return nc


_program_cache = {}


def _get_program():
    if "nc" not in _program_cache:
        _program_cache["nc"] = _build_program()
    return _program_cache["nc"]


def _pack_x(xs):
    # xs [N_SH, D_IN] f32 -> [P, NSL, 2, KC*HN] bf16
    a = xs.astype(bfloat16).reshape(NSL, 2, HN, KC, P)
    return np.ascontiguousarray(
        a.transpose(4, 0, 1, 3, 2)).reshape(P, NSL, 2, KC * HN)


def _pack_q(qs):
    # qs [O_SH, D_IN] i32 (values 0..255) -> (q-128)T packed [P, OT, KC*P] i8
    a = (qs.astype(np.uint8) ^ 0x80).view(np.int8)   # == q - 128, bit repack
    a = a.reshape(OT, P, KC, P)
    return np.ascontiguousarray(a.transpose(3, 0, 2, 1)).reshape(P, OT, KC * P)


def kernel(x, codes, basis_table, residual_q, residual_scales, bias):
    x = np.asarray(x, dtype=np.float32)
    codes = np.ascontiguousarray(np.asarray(codes, dtype=np.int32))
    basis_table = np.asarray(basis_table, dtype=np.float32)
    residual_q = np.asarray(residual_q, dtype=np.int32)
    residual_scales = np.ascontiguousarray(
        np.asarray(residual_scales, dtype=np.float32))
    bias = np.ascontiguousarray(np.asarray(bias, dtype=np.float32))

    x2 = x.reshape(B * S, D_IN)
    bp = np.ascontiguousarray(
        basis_table.astype(bfloat16).T.reshape(KC, P, BASIS).transpose(1, 0, 2))
    xpacks = {nb: _pack_x(x2[nb * N_SH:(nb + 1) * N_SH])
              for nb in range(N_SHARDS)}
    qpacks = {oc: _pack_q(residual_q[oc * O_SH:(oc + 1) * O_SH])
              for oc in range(O_SHARDS)}

    in_maps = []
    for core in range(N_CORES):
        oc, nb = divmod(core, N_SHARDS)
        osl = slice(oc * O_SH, (oc + 1) * O_SH)
        in_maps.append({
            "xp": xpacks[nb],
            "qp": qpacks[oc],
            "bp": bp,
            "codes_sh": np.ascontiguousarray(codes[osl]),
            "scales_sh": np.ascontiguousarray(residual_scales[osl]),
        })

    nc = _get_program()
    res = run_bass_kernel_spmd(nc, in_maps, core_ids=list(range(N_CORES)))

    y = np.empty((B * S, D_OUT), dtype=np.float32)
    for core in range(N_CORES):
        oc, nb = divmod(core, N_SHARDS)
        y[nb * N_SH:(nb + 1) * N_SH, oc * O_SH:(oc + 1) * O_SH] = \
            res.results[core]["y_sh"].astype(np.float32).T
    y += bias[None, :]
    return y.reshape(B, S, D_OUT)


# revision 14
# speedup vs baseline: 2.0060x; 1.3760x over previous
"""BitfieldLinear (vq_codebook) Trainium2 kernel — yT formulation, v3.

v3 vs v2:
- tokens sharded 8-way (no out-feature sharding): eliminates the
  duplicated z = basis @ xT work the 2-way o-sharding paid.
- q (16.8MB int8) streamed per o-block with a g-outer unit ordering:
  chunk g serves unit(0,g) and unit(1,g) back-to-back; the first WARM
  chunks are re-streamed for slice 1's tail. Deep (12-buf) fp8 chunk
  banking keeps the PE from ever waiting (the TRN2 PE clock ramps
  0.65->1.2->2.4GHz and needs 3us of gapless execution for full speed).
- r is decoded per-partition (f32) and folded into the P_bas PSUM
  evacuation scale instead of into the one-hot Sel, so Sel is a pure
  (idx==b) indicator and the startup ACT queue stays free for fp8
  weight-chunk casts.

y = x @ W^T + bias with W = r[:,None]*basis[idx] + s[:,None]*(q-128)/127.
Each core computes yT[o=4096, n=1024]:
  yT = (s/127) * P_res + r * P_bas          (+ bias on host)
  P_res[o, n] = sum_i (q[o,i]-128) x[n,i]   (fp8 DoubleRow matmuls)
  P_bas[o, n] = z[idx[o], n],  z[b, n] = sum_i basis[b,i] x[n,i]
Host ships transposed pre-packed operands (xT bf16, (q-128)T int8 — a
lossless bit repack — basisT bf16), so the device does no transposes
and no weight-decode pass.
"""

import numpy as np
from ml_dtypes import bfloat16

import concourse.bass as bass
import concourse.mybir as mybir
import concourse.tile as tile
from concourse.bass_utils import run_bass_kernel_spmd

# problem shape (hardcoded per harness contract)
B, S, D_IN, D_OUT, BASIS = 4, 2048, 4096, 4096, 256
N_CORES = 8
N_SHARDS = 8
O_SH = D_OUT                        # 4096 out-features per core (all)
N_SH = (B * S) // N_SHARDS          # 1024 token rows per core

P = 128
KC = D_IN // P                      # 32 contraction chunks
KP = KC // 2                        # 16 DoubleRow k-pairs
OT = O_SH // P                      # 32 o-blocks per core
NSL = 2                             # token slices per core
NW = N_SH // NSL                    # 512 tokens per slice
HN = NW // 2                        # 256-token half-slices for x loads
WARM = 8                            # o-blocks run un-paired at the start
LOOKAHEAD = 8                       # q chunks primed ahead of the PE

F32 = mybir.dt.float32
BF16 = mybir.dt.bfloat16
FP8 = mybir.dt.float8e4
I32 = mybir.dt.int32
I8 = mybir.dt.int8

_WAIT_LIMIT = 1


def _split_sync_waits(nc):
    """walrus in this container rejects instructions with more than one
    embedded sync-wait command; hoist the excess onto same-engine NoOps."""
    ctr = 0
    for f in nc.m.functions:
        for bb in f.blocks:
            new = []
            changed = False
            for inst in bb.instructions:
                si = inst.sync_info
                if si is not None and si.on_wait and len(si.on_wait) > _WAIT_LIMIT:
                    waits = list(si.on_wait)
                    excess, keep = waits[:-_WAIT_LIMIT], waits[-_WAIT_LIMIT:]
                    for i in range(0, len(excess), _WAIT_LIMIT):
                        ctr += 1
                        new.append(mybir.InstNoOp(
                            name=f"I-waitsplit-{ctr}",
                            engine=inst.engine,
                            ins=[], outs=[],
                            sync_info=mybir.SyncInfo(
                                on_wait=excess[i:i + _WAIT_LIMIT], on_update=[]),
                        ))
                    si.on_wait = keep
                    changed = True
                new.append(inst)
            if changed:
                bb.instructions = new


def _build_program(split_waits=True):
    nc = bass.Bass()
    Alu = mybir.AluOpType
    Act = mybir.ActivationFunctionType
    DR = mybir.MatmulPerfMode.DoubleRow

    # packed layouts (host-side):
    #   xp[p, ns, h, kc*HN + n] = x[ns*NW + h*HN + n, kc*P + p]   (bf16)
    #   qp[p, g, kc*P + o]      = q[g*P + o, kc*P + p] - 128      (int8)
    #   bp[p, kc, b]            = basis[b, kc*P + p]              (bf16)
    #   codes_pp[p, g]          = codes[g*P + p]; scales_sh likewise
    x_in = nc.dram_tensor("xp", [P, NSL, 2, KC * HN], BF16, kind="ExternalInput")
    q_in = nc.dram_tensor("qp", [P, OT, KC * P], I8, kind="ExternalInput")
    b_in = nc.dram_tensor("bp", [P, KC, BASIS], BF16, kind="ExternalInput")
    codes_in = nc.dram_tensor("codes_sh", [O_SH], I32, kind="ExternalInput")
    codes_pp_in = nc.dram_tensor("codes_pp", [P, OT], I32, kind="ExternalInput")
    scales_in = nc.dram_tensor("scales_sh", [P, OT], F32, kind="ExternalInput")
    y_out = nc.dram_tensor("y_sh", [O_SH, N_SH], BF16, kind="ExternalOutput")

    with tile.TileContext(nc) as tc:
        with (
            tc.tile_pool(name="const", bufs=1) as cpool,
            tc.tile_pool(name="rows4", bufs=3) as r4pool,   # [1, 2048] i32
            tc.tile_pool(name="xbf", bufs=2) as xbfpool,
            tc.tile_pool(name="x8", bufs=2) as x8pool,
            tc.tile_pool(name="qst", bufs=4) as qstpool,
            tc.tile_pool(name="q8", bufs=12) as q8pool,
            tc.tile_pool(name="zsb", bufs=2) as zsbpool,
            tc.tile_pool(name="rsb", bufs=3) as rsbpool,
            tc.tile_pool(name="y", bufs=4) as ypool,
            tc.tile_pool(name="psres", bufs=3, space="PSUM") as respool,
            tc.tile_pool(name="psbas", bufs=3, space="PSUM") as baspool,
            tc.tile_pool(name="psz", bufs=2, space="PSUM") as zpool,
        ):
            # ---- persistent tensors --------------------------------
            basisT = cpool.tile([P, KC, BASIS], BF16, name="basisT")
            sel_sb = [cpool.tile([P, 2, 512], BF16, name=f"sel{q}")
                      for q in range(O_SH // 512)]
            idx_row_f = cpool.tile([1, O_SH], BF16, name="idx_row")

            # ---- basisT load first: gates z0 on the PE -------------
            nc.scalar.dma_start(basisT[:], b_in[:])

            # ---- per-partition decode: r and s/127 -----------------
            codes_pp = cpool.tile([P, OT], I32, name="codes_pp")
            nc.sync.dma_start(codes_pp[:], codes_pp_in[:])
            rq_pp = cpool.tile([P, OT], I32, name="rq_pp")
            nc.vector.tensor_scalar(rq_pp[:], codes_pp[:], 8, None,
                                    Alu.logical_shift_right)
            r_pp = cpool.tile([P, OT], F32, name="r_pp")
            nc.scalar.activation(r_pp[:], rq_pp[:], Act.Copy,
                                 scale=1.0 / 65535.0)
            s_pp = cpool.tile([P, OT], F32)
            nc.sync.dma_start(s_pp[:], scales_in[:])
            sv_pp = cpool.tile([P, OT], F32)
            nc.vector.tensor_scalar_mul(sv_pp[:], s_pp[:], 1.0 / 127.0)

            # ---- idx rows (2048-wide halves) -----------------------
            for cd in range(2):
                cs = slice(cd * 2048, (cd + 1) * 2048)
                codes_row = r4pool.tile([1, 2048], I32, tag="r4",
                                        name=f"codes{cd}")
                nc.sync.dma_start(codes_row[:], codes_in[None, cs])
                idx_tmp = r4pool.tile([1, 2048], I32, tag="r4",
                                      name=f"idxt{cd}")
                nc.vector.tensor_scalar(idx_tmp[:], codes_row[:], 0xFF, None,
                                        Alu.bitwise_and)
                nc.scalar.activation(idx_row_f[:, cs], idx_tmp[:], Act.Copy)

            ones_row = cpool.tile([1, P], BF16)
            nc.vector.memset(ones_row[:], 1.0)
            ones_bc = cpool.tile([P, 512], BF16)
            nc.vector.memset(ones_bc[:], 1.0)

            iota_i = cpool.tile([P, 1], I32)
            nc.gpsimd.iota(iota_i[:], pattern=[[0, 1]], base=0,
                           channel_multiplier=1)
            iota_f = [cpool.tile([P, 1], F32, name=f"iota_f{bh}")
                      for bh in range(2)]
            nc.scalar.activation(iota_f[0][:], iota_i[:], Act.Copy)
            nc.scalar.activation(iota_f[1][:], iota_i[:], Act.Copy, bias=128.0,
                                 scale=1.0)

            # ---- q chunk stream: DMA (gpsimd) + fp8 cast (ACT) -----
            def q_chunk(g, it):
                qst = qstpool.tile([P, KC, P], I8, tag="qst",
                                   name=f"qst{g}_{it}")
                nc.gpsimd.dma_start(qst[:], q_in[:, g:g + 1, :])
                q8 = q8pool.tile([P, KC, P], FP8, tag="q8",
                                 name=f"q8_{g}_{it}")
                nc.scalar.copy(q8[:], qst[:])
                return q8

            # ---- one-hot Sel [128 b_lo, 2 b_hi, o]: (idx[o]==b) ----
            def sel_build(q):
                qs = slice(q * 512, (q + 1) * 512)
                pi = zpool.tile([P, 512], F32, tag="zt", name=f"pi{q}")
                nc.tensor.matmul(pi[:], lhsT=ones_row[:], rhs=idx_row_f[:, qs],
                                 start=True, stop=True)
                for bh in range(2):
                    nc.vector.scalar_tensor_tensor(
                        sel_sb[q][:, bh, :], pi[:], iota_f[bh][:, :1],
                        ones_bc[:], op0=Alu.is_equal, op1=Alu.mult)

            # ---- x loads + casts -----------------------------------
            xbf = {}     # (ns, h) -> bf16 half-slice tile
            x8 = {}      # ns -> fp8 slice tile

            def x_load(ns, h):
                t = xbfpool.tile([P, KC, HN], BF16, tag="xbf",
                                 name=f"xbf{ns}_{h}")
                eng = nc.sync if h == 0 else nc.scalar
                eng.dma_start(t[:], x_in[:, ns:ns + 1, h:h + 1, :])
                xbf[(ns, h)] = t

            def x_cast(ns):
                t = x8pool.tile([P, KC, NW], FP8, tag="x8", name=f"x8_{ns}")
                for h in range(2):
                    nc.vector.tensor_copy(t[:, :, h * HN:(h + 1) * HN],
                                          xbf[(ns, h)][:])
                x8[ns] = t

            # ---- z matmuls: z[b, n-slice] --------------------------
            def z_slice(ns):
                psz = [zpool.tile([P, NW], F32, tag="zt", name=f"psz{ns}_{bt}")
                       for bt in range(2)]
                for h in range(2):
                    hs = slice(h * HN, (h + 1) * HN)
                    for bt in range(2):
                        for kc in range(KC):
                            nc.tensor.matmul(
                                psz[bt][:, hs],
                                lhsT=basisT[:, kc, bt * P:(bt + 1) * P],
                                rhs=xbf[(ns, h)][:, kc, :],
                                start=(kc == 0), stop=(kc == KC - 1))
                z_sb = zsbpool.tile([P, 2, NW], BF16, tag="zsb",
                                    name=f"zsb{ns}")
                for bt in range(2):
                    nc.vector.tensor_copy(z_sb[:, bt, :], psz[bt][:])
                return z_sb

            # ---- main unit: yT[o-block g, n-slice ns] --------------
            def unit(ns, g, q8, z_sb):
                ps = respool.tile([P, NW], F32, tag="res", name=f"ps{ns}_{g}")
                for kp in range(KP):
                    nc.tensor.matmul(ps[:],
                                     lhsT=q8[:, 2 * kp:2 * kp + 2, :],
                                     rhs=x8[ns][:, 2 * kp:2 * kp + 2, :],
                                     start=(kp == 0), stop=(kp == KP - 1),
                                     perf_mode=DR)
                pb = baspool.tile([P, NW], F32, tag="bas", name=f"pb{ns}_{g}")
                for bt in range(2):
                    nc.tensor.matmul(pb[:],
                                     lhsT=sel_sb[g // 4][:, bt,
                                                         (g % 4) * P:
                                                         (g % 4 + 1) * P],
                                     rhs=z_sb[:, bt, :],
                                     start=(bt == 0), stop=(bt == 1))
                # yT = (s/127) * ps + r * pb
                r_sb = rsbpool.tile([P, NW], BF16, tag="rsb",
                                    name=f"rs{ns}_{g}")
                nc.scalar.activation(r_sb[:], pb[:], Act.Copy,
                                     scale=r_pp[:, g:g + 1])
                y_t = ypool.tile([P, NW], BF16, tag="y", name=f"y{ns}_{g}")
                nc.vector.scalar_tensor_tensor(
                    y_t[:], ps[:], sv_pp[:, g:g + 1], r_sb[:],
                    op0=Alu.mult, op1=Alu.add)
                yeng = nc.sync if g % 2 == 0 else nc.gpsimd
                yeng.dma_start(
                    y_out[g * P:(g + 1) * P, ns * NW:(ns + 1) * NW], y_t[:])

            # ---- emission -----------------------------------------
            for ns in range(NSL):
                for h in range(2):
                    x_load(ns, h)
            for ns in range(NSL):
                x_cast(ns)

            chunk_seq = list(range(OT)) + list(range(WARM))
            q8_tiles = {}
            state = {"emitted": 0}

            def emit_chunks(n):
                for _ in range(n):
                    e = state["emitted"]
                    if e < len(chunk_seq):
                        q8_tiles[e] = q_chunk(chunk_seq[e], 0 if e < OT else 1)
                        state["emitted"] = e + 1

            emit_chunks(LOOKAHEAD)
            for q in range(O_SH // 512):
                sel_build(q)
            zs = {0: z_slice(0)}
            ci = 0
            for g in range(WARM):
                emit_chunks(1)
                unit(0, g, q8_tiles[ci], zs[0])
                ci += 1
            zs[1] = z_slice(1)
            for g in range(WARM, OT):
                emit_chunks(1)
                q8 = q8_tiles[ci]
                ci += 1
                unit(0, g, q8, zs[0])
                unit(1, g, q8, zs[1])
            for g in range(WARM):
                emit_chunks(1)
                unit(1, g, q8_tiles[ci], zs[1])
                ci += 1

    if split_waits:
        _split_sync_waits(nc)
    return nc


_program_cache = {}


def _get_program():
    if "nc" not in _program_cache:
        _program_cache["nc"] = _build_program()
    return _program_cache["nc"]


def _pack_x(xs):
    # xs [N_SH, D_IN] f32 -> [P, NSL, 2, KC*HN] bf16
    a = xs.astype(bfloat16).reshape(NSL, 2, HN, KC, P)
    return np.ascontiguousarray(
        a.transpose(4, 0, 1, 3, 2)).reshape(P, NSL, 2, KC * HN)


def _pack_q(qs):
    # qs [O_SH, D_IN] i32 (values 0..255) -> (q-128)T packed [P, OT, KC*P] i8
    a = (qs.astype(np.uint8) ^ 0x80).view(np.int8)   # == q - 128, bit repack
    a = a.reshape(OT, P, KC, P)
    return np.ascontiguousarray(a.transpose(3, 0, 2, 1)).reshape(P, OT, KC * P)


def kernel(x, codes, basis_table, residual_q, residual_scales, bias):
    x = np.asarray(x, dtype=np.float32)
    codes = np.ascontiguousarray(np.asarray(codes, dtype=np.int32))
    basis_table = np.asarray(basis_table, dtype=np.float32)
    residual_q = np.asarray(residual_q, dtype=np.int32)
    residual_scales = np.ascontiguousarray(
        np.asarray(residual_scales, dtype=np.float32))
    bias = np.ascontiguousarray(np.asarray(bias, dtype=np.float32))

    x2 = x.reshape(B * S, D_IN)
    bp = np.ascontiguousarray(
        basis_table.astype(bfloat16).T.reshape(KC, P, BASIS).transpose(1, 0, 2))
    qp = _pack_q(residual_q)
    sp = np.ascontiguousarray(residual_scales.reshape(OT, P).T)
    cp = np.ascontiguousarray(codes.reshape(OT, P).T)

    in_maps = []
    for core in range(N_CORES):
        in_maps.append({
            "xp": _pack_x(x2[core * N_SH:(core + 1) * N_SH]),
            "qp": qp,
            "bp": bp,
            "codes_sh": codes,
            "codes_pp": cp,
            "scales_sh": sp,
        })

    nc = _get_program()
    res = run_bass_kernel_spmd(nc, in_maps, core_ids=list(range(N_CORES)))

    y = np.empty((B * S, D_OUT), dtype=np.float32)
    for core in range(N_CORES):
        y[core * N_SH:(core + 1) * N_SH] = \
            res.results[core]["y_sh"].astype(np.float32).T
    y += bias[None, :]
    return y.reshape(B, S, D_OUT)


# revision 17
# speedup vs baseline: 2.0333x; 1.0136x over previous
"""BitfieldLinear (vq_codebook) Trainium2 kernel — yT formulation, v3.

v3 vs v2:
- tokens sharded 8-way (no out-feature sharding): eliminates the
  duplicated z = basis @ xT work the 2-way o-sharding paid.
- q (16.8MB int8) streamed per o-block with a g-outer unit ordering:
  chunk g serves unit(0,g) and unit(1,g) back-to-back; the first WARM
  chunks are re-streamed for slice 1's tail. Deep (12-buf) fp8 chunk
  banking keeps the PE from ever waiting (the TRN2 PE clock ramps
  0.65->1.2->2.4GHz and needs 3us of gapless execution for full speed).
- r is decoded per-partition (f32) and folded into the P_bas PSUM
  evacuation scale instead of into the one-hot Sel, so Sel is a pure
  (idx==b) indicator and the startup ACT queue stays free for fp8
  weight-chunk casts.

y = x @ W^T + bias with W = r[:,None]*basis[idx] + s[:,None]*(q-128)/127.
Each core computes yT[o=4096, n=1024]:
  yT = (s/127) * P_res + r * P_bas          (+ bias on host)
  P_res[o, n] = sum_i (q[o,i]-128) x[n,i]   (fp8 DoubleRow matmuls)
  P_bas[o, n] = z[idx[o], n],  z[b, n] = sum_i basis[b,i] x[n,i]
Host ships transposed pre-packed operands (xT bf16, (q-128)T int8 — a
lossless bit repack — basisT bf16), so the device does no transposes
and no weight-decode pass.
"""

import numpy as np
from ml_dtypes import bfloat16

import concourse.bass as bass
import concourse.mybir as mybir
import concourse.tile as tile
from concourse.bass_utils import run_bass_kernel_spmd

# problem shape (hardcoded per harness contract)
B, S, D_IN, D_OUT, BASIS = 4, 2048, 4096, 4096, 256
N_CORES = 8
N_SHARDS = 8
O_SH = D_OUT                        # 4096 out-features per core (all)
N_SH = (B * S) // N_SHARDS          # 1024 token rows per core

P = 128
KC = D_IN // P                      # 32 contraction chunks
KP = KC // 2                        # 16 DoubleRow k-pairs
OT = O_SH // P                      # 32 o-blocks per core
NSL = 2                             # token slices per core
NW = N_SH // NSL                    # 512 tokens per slice
HN = NW // 2                        # 256-token half-slices for x loads
WARM = 8                            # o-blocks run un-paired at the start
LOOKAHEAD = 8                       # q chunks primed ahead of the PE

F32 = mybir.dt.float32
BF16 = mybir.dt.bfloat16
FP8 = mybir.dt.float8e4
I32 = mybir.dt.int32
I8 = mybir.dt.int8

_WAIT_LIMIT = 1


def _split_sync_waits(nc):
    """walrus in this container rejects instructions with more than one
    embedded sync-wait command; hoist the excess onto same-engine NoOps."""
    ctr = 0
    for f in nc.m.functions:
        for bb in f.blocks:
            new = []
            changed = False
            for inst in bb.instructions:
                si = inst.sync_info
                if si is not None and si.on_wait and len(si.on_wait) > _WAIT_LIMIT:
                    waits = list(si.on_wait)
                    excess, keep = waits[:-_WAIT_LIMIT], waits[-_WAIT_LIMIT:]
                    for i in range(0, len(excess), _WAIT_LIMIT):
                        ctr += 1
                        new.append(mybir.InstNoOp(
                            name=f"I-waitsplit-{ctr}",
                            engine=inst.engine,
                            ins=[], outs=[],
                            sync_info=mybir.SyncInfo(
                                on_wait=excess[i:i + _WAIT_LIMIT], on_update=[]),
                        ))
                    si.on_wait = keep
                    changed = True
                new.append(inst)
            if changed:
                bb.instructions = new


def _build_program(split_waits=True):
    nc = bass.Bass()
    Alu = mybir.AluOpType
    Act = mybir.ActivationFunctionType
    DR = mybir.MatmulPerfMode.DoubleRow

    # packed layouts (host-side):
    #   xp[p, ns, h, kc*HN + n] = x[ns*NW + h*HN + n, kc*P + p]   (bf16)
    #   qp[p, g, kc*P + o]      = q[g*P + o, kc*P + p] - 128      (int8)
    #   bp[p, kc, b]            = basis[b, kc*P + p]              (bf16)
    #   codes_pp[p, g]          = codes[g*P + p]; scales_sh likewise
    x_in = nc.dram_tensor("xp", [P, NSL, 2, KC * HN], BF16, kind="ExternalInput")
    q_in = nc.dram_tensor("qp", [P, OT, KC * P], I8, kind="ExternalInput")
    b_in = nc.dram_tensor("bp", [P, KC, BASIS], BF16, kind="ExternalInput")
    codes_in = nc.dram_tensor("codes_sh", [O_SH], I32, kind="ExternalInput")
    codes_pp_in = nc.dram_tensor("codes_pp", [P, OT], I32, kind="ExternalInput")
    scales_in = nc.dram_tensor("scales_sh", [P, OT], F32, kind="ExternalInput")
    y_out = nc.dram_tensor("y_sh", [O_SH, N_SH], BF16, kind="ExternalOutput")

    with tile.TileContext(nc) as tc:
        with (
            tc.tile_pool(name="const", bufs=1) as cpool,
            tc.tile_pool(name="rows4", bufs=3) as r4pool,   # [1, 2048] i32
            tc.tile_pool(name="xbf", bufs=2) as xbfpool,
            tc.tile_pool(name="x8", bufs=2) as x8pool,
            tc.tile_pool(name="qst", bufs=4) as qstpool,
            tc.tile_pool(name="q8", bufs=12) as q8pool,
            tc.tile_pool(name="zsb", bufs=2) as zsbpool,
            tc.tile_pool(name="rsb", bufs=3) as rsbpool,
            tc.tile_pool(name="y", bufs=4) as ypool,
            tc.tile_pool(name="psres", bufs=3, space="PSUM") as respool,
            tc.tile_pool(name="psbas", bufs=3, space="PSUM") as baspool,
            tc.tile_pool(name="psz", bufs=2, space="PSUM") as zpool,
        ):
            # ---- persistent tensors --------------------------------
            basisT = cpool.tile([P, KC, BASIS], BF16, name="basisT")
            sel_sb = [cpool.tile([P, 2, 512], BF16, name=f"sel{q}")
                      for q in range(O_SH // 512)]
            idx_row_f = cpool.tile([1, O_SH], BF16, name="idx_row")

            # ---- basisT load first on the scalar ring --------------
            # the [128, OT] constant loads are 128 tiny descriptors each
            # (~10us of DMA-ring time); they ride the scalar ring BEHIND
            # basisT so the sync ring serves codes rows + x immediately.
            nc.scalar.dma_start(basisT[:], b_in[:])
            codes_pp = cpool.tile([P, OT], I32, name="codes_pp")
            nc.scalar.dma_start(codes_pp[:], codes_pp_in[:])
            s_pp = cpool.tile([P, OT], F32)
            nc.scalar.dma_start(s_pp[:], scales_in[:])

            # per-partition decode on DVE (keeps the ACT queue free for
            # the fp8 weight-chunk casts): r = (codes>>8)/65535, s/127
            rq_pp = cpool.tile([P, OT], I32, name="rq_pp")
            nc.vector.tensor_scalar(rq_pp[:], codes_pp[:], 8, None,
                                    Alu.logical_shift_right)
            r_pp = cpool.tile([P, OT], F32, name="r_pp")
            nc.vector.tensor_scalar_mul(r_pp[:], rq_pp[:], 1.0 / 65535.0)
            sv_pp = cpool.tile([P, OT], F32)
            nc.vector.tensor_scalar_mul(sv_pp[:], s_pp[:], 1.0 / 127.0)

            # ---- idx rows (2048-wide halves) -----------------------
            for cd in range(2):
                cs = slice(cd * 2048, (cd + 1) * 2048)
                codes_row = r4pool.tile([1, 2048], I32, tag="r4",
                                        name=f"codes{cd}")
                nc.sync.dma_start(codes_row[:], codes_in[None, cs])
                idx_tmp = r4pool.tile([1, 2048], I32, tag="r4",
                                      name=f"idxt{cd}")
                nc.vector.tensor_scalar(idx_tmp[:], codes_row[:], 0xFF, None,
                                        Alu.bitwise_and)
                nc.scalar.activation(idx_row_f[:, cs], idx_tmp[:], Act.Copy)

            ones_row = cpool.tile([1, P], BF16)
            nc.vector.memset(ones_row[:], 1.0)
            ones_bc = cpool.tile([P, 512], BF16)
            nc.vector.memset(ones_bc[:], 1.0)

            iota_i = cpool.tile([P, 1], I32)
            nc.gpsimd.iota(iota_i[:], pattern=[[0, 1]], base=0,
                           channel_multiplier=1)
            iota_f = [cpool.tile([P, 1], F32, name=f"iota_f{bh}")
                      for bh in range(2)]
            nc.scalar.activation(iota_f[0][:], iota_i[:], Act.Copy)
            nc.scalar.activation(iota_f[1][:], iota_i[:], Act.Copy, bias=128.0,
                                 scale=1.0)

            # ---- q chunk stream: DMA (gpsimd) + fp8 cast (ACT) -----
            def q_chunk(g, it):
                qst = qstpool.tile([P, KC, P], I8, tag="qst",
                                   name=f"qst{g}_{it}")
                nc.gpsimd.dma_start(qst[:], q_in[:, g:g + 1, :])
                q8 = q8pool.tile([P, KC, P], FP8, tag="q8",
                                 name=f"q8_{g}_{it}")
                nc.scalar.copy(q8[:], qst[:])
                return q8

            # ---- one-hot Sel [128 b_lo, 2 b_hi, o]: (idx[o]==b) ----
            # pi rides the res PSUM pool so the z pool is free for both
            # slices' psz tiles during the z-first phase.
            def sel_build(q):
                qs = slice(q * 512, (q + 1) * 512)
                pi = respool.tile([P, 512], F32, tag="res", name=f"pi{q}")
                nc.tensor.matmul(pi[:], lhsT=ones_row[:], rhs=idx_row_f[:, qs],
                                 start=True, stop=True)
                for bh in range(2):
                    nc.vector.scalar_tensor_tensor(
                        sel_sb[q][:, bh, :], pi[:], iota_f[bh][:, :1],
                        ones_bc[:], op0=Alu.is_equal, op1=Alu.mult)

            # ---- x loads + casts -----------------------------------
            xbf = {}     # (ns, h) -> bf16 half-slice tile
            x8 = {}      # ns -> fp8 slice tile

            def x_load(ns, h):
                t = xbfpool.tile([P, KC, HN], BF16, tag="xbf",
                                 name=f"xbf{ns}_{h}")
                eng = nc.sync if h == 0 else nc.scalar
                eng.dma_start(t[:], x_in[:, ns:ns + 1, h:h + 1, :])
                xbf[(ns, h)] = t

            def x_cast(ns):
                t = x8pool.tile([P, KC, NW], FP8, tag="x8", name=f"x8_{ns}")
                for h in range(2):
                    nc.vector.tensor_copy(t[:, :, h * HN:(h + 1) * HN],
                                          xbf[(ns, h)][:])
                x8[ns] = t

            # ---- z matmuls: z[b, n-slice] --------------------------
            def z_slice(ns):
                psz = [zpool.tile([P, NW], F32, tag="zt", name=f"psz{ns}_{bt}")
                       for bt in range(2)]
                for h in range(2):
                    hs = slice(h * HN, (h + 1) * HN)
                    for bt in range(2):
                        for kc in range(KC):
                            nc.tensor.matmul(
                                psz[bt][:, hs],
                                lhsT=basisT[:, kc, bt * P:(bt + 1) * P],
                                rhs=xbf[(ns, h)][:, kc, :],
                                start=(kc == 0), stop=(kc == KC - 1))
                z_sb = zsbpool.tile([P, 2, NW], BF16, tag="zsb",
                                    name=f"zsb{ns}")
                for bt in range(2):
                    nc.vector.tensor_copy(z_sb[:, bt, :], psz[bt][:])
                return z_sb

            # ---- main unit: yT[o-block g, n-slice ns] --------------
            def unit(ns, g, q8, z_sb):
                ps = respool.tile([P, NW], F32, tag="res", name=f"ps{ns}_{g}")
                for kp in range(KP):
                    nc.tensor.matmul(ps[:],
                                     lhsT=q8[:, 2 * kp:2 * kp + 2, :],
                                     rhs=x8[ns][:, 2 * kp:2 * kp + 2, :],
                                     start=(kp == 0), stop=(kp == KP - 1),
                                     perf_mode=DR)
                pb = baspool.tile([P, NW], F32, tag="bas", name=f"pb{ns}_{g}")
                for bt in range(2):
                    nc.tensor.matmul(pb[:],
                                     lhsT=sel_sb[g // 4][:, bt,
                                                         (g % 4) * P:
                                                         (g % 4 + 1) * P],
                                     rhs=z_sb[:, bt, :],
                                     start=(bt == 0), stop=(bt == 1))
                # yT = (s/127) * ps + r * pb
                r_sb = rsbpool.tile([P, NW], BF16, tag="rsb",
                                    name=f"rs{ns}_{g}")
                nc.scalar.activation(r_sb[:], pb[:], Act.Copy,
                                     scale=r_pp[:, g:g + 1])
                y_t = ypool.tile([P, NW], BF16, tag="y", name=f"y{ns}_{g}")
                nc.vector.scalar_tensor_tensor(
                    y_t[:], ps[:], sv_pp[:, g:g + 1], r_sb[:],
                    op0=Alu.mult, op1=Alu.add)
                yeng = nc.sync if g % 2 == 0 else nc.gpsimd
                yeng.dma_start(
                    y_out[g * P:(g + 1) * P, ns * NW:(ns + 1) * NW], y_t[:])

            # ---- emission: z-first, all units paired ---------------
            for ns in range(NSL):
                for h in range(2):
                    x_load(ns, h)
            x_cast(0)

            q8_tiles = {}
            state = {"emitted": 0}

            def emit_chunks(n):
                for _ in range(n):
                    e = state["emitted"]
                    if e < OT:
                        q8_tiles[e] = q_chunk(e, 0)
                        state["emitted"] = e + 1

            emit_chunks(LOOKAHEAD)
            for q in range(O_SH // 512):
                sel_build(q)
            x_cast(1)
            zs = {0: z_slice(0), 1: z_slice(1)}
            for g in range(OT):
                emit_chunks(1)
                q8 = q8_tiles[g]
                unit(0, g, q8, zs[0])
                unit(1, g, q8, zs[1])

    if split_waits:
        _split_sync_waits(nc)
    return nc


_program_cache = {}


def _get_program():
    if "nc" not in _program_cache:
        _program_cache["nc"] = _build_program()
    return _program_cache["nc"]


def _pack_x(xs):
    # xs [N_SH, D_IN] f32 -> [P, NSL, 2, KC*HN] bf16
    a = xs.astype(bfloat16).reshape(NSL, 2, HN, KC, P)
    return np.ascontiguousarray(
        a.transpose(4, 0, 1, 3, 2)).reshape(P, NSL, 2, KC * HN)


def _pack_q(qs):
    # qs [O_SH, D_IN] i32 (values 0..255) -> (q-128)T packed [P, OT, KC*P] i8
    a = (qs.astype(np.uint8) ^ 0x80).view(np.int8)   # == q - 128, bit repack
    a = a.reshape(OT, P, KC, P)
    return np.ascontiguousarray(a.transpose(3, 0, 2, 1)).reshape(P, OT, KC * P)


def kernel(x, codes, basis_table, residual_q, residual_scales, bias):
    x = np.asarray(x, dtype=np.float32)
    codes = np.ascontiguousarray(np.asarray(codes, dtype=np.int32))
    basis_table = np.asarray(basis_table, dtype=np.float32)
    residual_q = np.asarray(residual_q, dtype=np.int32)
    residual_scales = np.ascontiguousarray(
        np.asarray(residual_scales, dtype=np.float32))
    bias = np.ascontiguousarray(np.asarray(bias, dtype=np.float32))

    x2 = x.reshape(B * S, D_IN)
    bp = np.ascontiguousarray(
        basis_table.astype(bfloat16).T.reshape(KC, P, BASIS).transpose(1, 0, 2))
    qp = _pack_q(residual_q)
    sp = np.ascontiguousarray(residual_scales.reshape(OT, P).T)
    cp = np.ascontiguousarray(codes.reshape(OT, P).T)

    in_maps = []
    for core in range(N_CORES):
        in_maps.append({
            "xp": _pack_x(x2[core * N_SH:(core + 1) * N_SH]),
            "qp": qp,
            "bp": bp,
            "codes_sh": codes,
            "codes_pp": cp,
            "scales_sh": sp,
        })

    nc = _get_program()
    res = run_bass_kernel_spmd(nc, in_maps, core_ids=list(range(N_CORES)))

    y = np.empty((B * S, D_OUT), dtype=np.float32)
    for core in range(N_CORES):
        y[core * N_SH:(core + 1) * N_SH] = \
            res.results[core]["y_sh"].astype(np.float32).T
    y += bias[None, :]
    return y.reshape(B, S, D_OUT)
